# revision 1
# baseline (speedup 1.0000x reference)
"""BP-MLL loss kernel for Trainium2 (Bass/Tile), data-parallel over 8 NeuronCores.

Reference computation (per row r of [B, L] inputs):
    s_pos[r] = sum_{j: t=1} exp(-x[r,j])
    s_neg[r] = sum_{j: t=0} exp( x[r,j])
    n_pos[r] = #{j: t=1},  n_neg[r] = L - n_pos[r]
    loss     = sum_r s_pos[r]*s_neg[r] / (n_pos[r]*n_neg[r])

Sharding: batch dim B=8192 split 8 ways (1024 rows/core); each core computes a
scalar partial loss on-device; host sums the 8 partials.

Per-core device plan. The 0/1 mask is folded into the exp arguments so each
tile [128 rows, F cols] (rows on partitions) needs one DVE pass and two ACT
passes, each with a fused free-axis accumulation:
    DVE:  u = C*t - x  (C = 8192 = 2^13)      accum -> su ~= C*n_pos
    ACT:  exp(u - C) = exp(-x) if t=1 else 0  accum -> s_pos
    ACT:  exp(-u)    = exp(x)  if t=0 else 0  accum -> s_neg
C is a power of 2, so C*t and the C*n_pos part of the accumulator are exact;
the -sum(x) and rounding perturbations in su are O(300) << C/2, so
n_pos = su/C is accurate to ~0.03 counts (~1e-6 relative in n_pos*n_neg).
fl(C - x) costs x half an ulp of C (2^-11): ~1e-5 relative noise in s_pos,
zero-mean across a row. exp(-C...) flushes cleanly to 0.

Per row-group epilogue (overlaps the stream): combine chunk partials, per-row
loss terms, and a (-C^2)-weighted ones-matmul accumulated across row groups in
one PSUM bank; -C^2 folds the partition-reduce sign and the /C^2 from
denom' = (su - C*L)*su = -C^2*n_pos*n_neg.

DMA: io_bufs=6 tile pairs in flight - queue depth is what keeps all 16 SDMA
engines saturated (~420 GB/s/core best case). The last chunk is tapered into
small pieces so the post-stream serial compute tail is short.
"""

import numpy as np

import concourse.bacc as bacc
import concourse.bass as bass
import concourse.tile as tile
from concourse import mybir
from concourse.bass_utils import run_bass_kernel_spmd

F32 = mybir.dt.float32
I32 = mybir.dt.int32
AF = mybir.ActivationFunctionType
ALU = mybir.AluOpType

B, L = 8192, 10000
N_CORES = 8
ROWS = B // N_CORES  # rows per core
P = 128
BIG = 8192.0  # mask scale: power of 2; exp(-8192) flushes to 0,
# and n_pos = su/BIG is recoverable since |sum(x)| << BIG


def build_bass(
    rows=ROWS,
    cols=L,
    f_c=2500,
    io_bufs=6,
    u_bufs=3,
    taper=(1250, 625, 625),  # replaces the final f_c-wide chunk
    t_via_gpsimd=False,  # issue t loads on the SWDGE ring (2nd descriptor queue)
    dma_only=False,
):
    """Build the per-core Bass program. Same program runs SPMD on all cores."""
    assert rows % P == 0 and cols % f_c == 0
    n_rg = rows // P
    n_ch = cols // f_c
    if taper is not None:
        assert sum(taper) == f_c

    # per row group: list of (col_offset, width)
    widths = [f_c] * n_ch
    last_widths = widths[:-1] + (list(taper) if taper else [f_c])

    def chunks_for(rg):
        ws = last_widths if rg == n_rg - 1 else widths
        offs = np.concatenate([[0], np.cumsum(ws)[:-1]]).tolist()
        return list(zip(offs, ws))

    n_slots = sum(len(chunks_for(rg)) for rg in range(n_rg))

    nc = bacc.Bacc("TRN2", target_bir_lowering=False, debug=False)
    x = nc.dram_tensor("x", [rows, cols], F32, kind="ExternalInput").ap()
    t = nc.dram_tensor("t", [rows, cols], I32, kind="ExternalInput").ap()
    out = nc.dram_tensor("out", [1, 1], F32, kind="ExternalOutput").ap()

    with tile.TileContext(nc) as tc:
        with (
            tc.tile_pool(name="io", bufs=io_bufs) as io_pool,
            tc.tile_pool(name="upool", bufs=u_bufs) as u_pool,
            tc.tile_pool(name="epool", bufs=2) as e_pool,
            tc.tile_pool(name="acc", bufs=1) as acc_pool,
            tc.tile_pool(name="small", bufs=1) as small_pool,
            tc.tile_pool(name="psum", bufs=1, space="PSUM") as psum_pool,
        ):
            acc_spos = acc_pool.tile([P, n_slots], F32, tag="acc_spos")
            acc_sneg = acc_pool.tile([P, n_slots], F32, tag="acc_sneg")
            acc_su = acc_pool.tile([P, n_slots], F32, tag="acc_su")

            if not dma_only:
                neg_big = acc_pool.tile([P, 1], F32, tag="neg_big")
                nc.vector.memset(neg_big[:], -BIG)
                w = acc_pool.tile([P, 1], F32, tag="w")
                nc.vector.memset(w[:], -(BIG * BIG))
                ps = psum_pool.tile([1, 1], F32, tag="ps")

            sl = 0
            for rg in range(n_rg):
                r0 = rg * P
                rg_chunks = chunks_for(rg)
                s0 = sl
                for c0, fw in rg_chunks:
                    xt = io_pool.tile([P, fw], F32, tag="x")
                    tt = io_pool.tile([P, fw], I32, tag="t")
                    nc.sync.dma_start(xt[:], x[r0 : r0 + P, c0 : c0 + fw])
                    t_eng = nc.gpsimd if t_via_gpsimd else nc.sync
                    t_eng.dma_start(tt[:], t[r0 : r0 + P, c0 : c0 + fw])
                    if dma_only:
                        sl += 1
                        continue

                    ut = u_pool.tile([P, fw], F32, tag="u")
                    # u = C*t - x ; accum -> su ~= C*n_pos
                    nc.vector.scalar_tensor_tensor(
                        ut[:],
                        tt[:],
                        BIG,
                        xt[:],
                        op0=ALU.mult,
                        op1=ALU.subtract,
                        accum_out=acc_su[:, sl : sl + 1],
                    )
                    ea = e_pool.tile([P, fw], F32, tag="escr")
                    # exp(u - C): t=1 -> exp(-x); t=0 -> 0
                    nc.scalar.activation(
                        ea[:],
                        ut[:],
                        AF.Exp,
                        bias=neg_big[:],
                        scale=1.0,
                        accum_out=acc_spos[:, sl : sl + 1],
                    )
                    eb = e_pool.tile([P, fw], F32, tag="escr")
                    # exp(-u): t=0 -> exp(x); t=1 -> 0
                    nc.scalar.activation(
                        eb[:],
                        ut[:],
                        AF.Exp,
                        scale=-1.0,
                        accum_out=acc_sneg[:, sl : sl + 1],
                    )
                    sl += 1

                if dma_only:
                    continue

                # --- per-row-group epilogue (overlaps later chunks' stream) ---
                s1 = sl
                s_pos = small_pool.tile([P, 1], F32, tag="s_pos")
                s_neg = small_pool.tile([P, 1], F32, tag="s_neg")
                su = small_pool.tile([P, 1], F32, tag="su")
                for dst, src in (
                    (s_pos, acc_spos),
                    (s_neg, acc_sneg),
                    (su, acc_su),
                ):
                    nc.vector.tensor_reduce(
                        dst[:],
                        src[:, s0:s1],
                        axis=mybir.AxisListType.X,
                        op=ALU.add,
                    )
                numer = small_pool.tile([P, 1], F32, tag="numer")
                nc.vector.tensor_tensor(numer[:], s_pos[:], s_neg[:], op=ALU.mult)
                # denom' = (su - C*L) * su = -C^2 * n_pos * n_neg  (su = C*n_pos)
                denom = small_pool.tile([P, 1], F32, tag="denom")
                nc.vector.scalar_tensor_tensor(
                    denom[:],
                    su[:],
                    BIG * float(cols),
                    su[:],
                    op0=ALU.subtract,
                    op1=ALU.mult,
                )
                recip = small_pool.tile([P, 1], F32, tag="recip")
                nc.vector.reciprocal(recip[:], denom[:])
                contrib = small_pool.tile([P, 1], F32, tag="contrib")
                nc.vector.tensor_tensor(
                    contrib[:], numer[:], recip[:], op=ALU.mult
                )
                # PSUM accumulate across row groups:
                # ps += (-900 ones)^T @ contrib = sum_p numer/(n_pos*n_neg)
                nc.tensor.matmul(
                    ps[:],
                    w[:],
                    contrib[:],
                    start=(rg == 0),
                    stop=(rg == n_rg - 1),
                )

            res = small_pool.tile([1, 1], F32, tag="res")
            if dma_only:
                nc.vector.memset(res[:], 0.0)
            else:
                nc.vector.tensor_copy(res[:], ps[:])
            nc.sync.dma_start(out[0:1, 0:1], res[:])

    nc.compile()
    return nc


_NC_CACHE = {}


def _get_nc():
    if "nc" not in _NC_CACHE:
        _NC_CACHE["nc"] = build_bass()
    return _NC_CACHE["nc"]


def kernel(input, target):
    x = np.ascontiguousarray(np.asarray(input, dtype=np.float32))
    t = np.ascontiguousarray(np.asarray(target, dtype=np.int32))
    assert x.shape == (B, L) and t.shape == (B, L)

    nc = _get_nc()
    in_maps = [
        {
            "x": x[i * ROWS : (i + 1) * ROWS],
            "t": t[i * ROWS : (i + 1) * ROWS],
        }
        for i in range(N_CORES)
    ]
    res = run_bass_kernel_spmd(nc, in_maps, core_ids=list(range(N_CORES)))
    partials = np.array(
        [res.results[i]["out"][0, 0] for i in range(N_CORES)], dtype=np.float64
    )
    return np.float32(partials.sum())

